# revision 28
# baseline (speedup 1.0000x reference)
# Multi-head attention (B=2, N=2048, C=1024, H=16) on 8 trn2 NeuronCores.
#
# Sharding: core = (batch b = core//4, head-group hg = core%4, 4 heads each).
# Each core computes qkv/attention/proj for its 4 heads of its batch and
# returns a partial projection output [N, C]; the host sums the 4 partials
# per batch and adds b_proj.
#
# Final structure (v7):
#   - x is transposed on the HOST and fed as bf16 xT [C, N]; w_qkv also
#     bf16 -> qkv matmuls are all-bf16 (FWL weight loads, half the DMA).
#   - S^T matmuls contract K=64 with the two heads of a pair packed at
#     partitions 0:64 / 64:128; tile_position row-tiling (auto-inferred)
#     lets the PE run both heads' S matmuls concurrently.
#   - exp runs on ACT only (hard floor ~130us at 1 col/cycle); S tiles are
#     computed as early as dependencies allow (interleaved into the qkv
#     phase) and bf16 E^T tiles hoarded in SBUF (~48 tiles) so ACT is
#     saturated from ~15us in. All PSUM->SBUF drains go to DVE.
#   - E-tile pools are ring buffers in iq-major order; attention passes
#     run q2 (512-wide query quarters) OUTER so each quarter's O
#     accumulator [65, 512] finishes early and its normalize (DVE recip +
#     gpsimd partition-broadcast + multiply from PSUM) hides under the
#     next quarter's matmuls. psO quarters are single banks (bufs=4).
#   - exp is mixed-precision: tiles hoarded during the qkv phase are bf16
#     (slower ACT write, but scope-A ACT has slack and bf16 doubles the
#     hoard capacity); just-in-time tiles during attention are f32r
#     (~1.0us vs ~1.25us per [128,1024] tile). v' is kept in both dtypes.
#   - proj (bf16 OT x bf16 wp) follows immediately; dummy matmuls gated on
#     the last normalize chain keep the PE HAM-warm into proj; output is
#     written bf16 (halves out-DMA) and summed across cores in f32 on host.
import sys

import numpy as np

if "/opt/trn_rl_repo" not in sys.path:
    sys.path.insert(0, "/opt/trn_rl_repo")

B, NSEQ, C = 2, 2048, 1024
H, HD = 16, 64
P = 128
SCALE = HD**-0.5

_cache = {}


def _build(nseq):
    from contextlib import ExitStack

    import concourse.tile as tile
    from concourse import bacc, mybir

    f32 = mybir.dt.float32
    f32r = mybir.dt.float32r
    bf16 = mybir.dt.bfloat16
    EXP = mybir.ActivationFunctionType.Exp

    NJT = nseq // P            # key tiles (128 each)
    NIT = nseq // P            # query tiles for proj
    QCH = 512                  # qkv seq chunk
    NCH = nseq // QCH          # 4
    ECH = 512                  # proj output chunk

    # E^T tile pools (bf16 [128, 1024] = 2KB/partition each, one per
    # (pair, jt, iq) covering both units), keyed (pair, ihalf).
    ET_BUFS = {(0, 0): 26, (0, 1): 6, (1, 0): 14, (1, 1): 6}
    POOL_ORDER = [(0, 0), (0, 1), (1, 0), (1, 1)]

    nc = bacc.Bacc("TRN2", target_bir_lowering=False, debug=False, num_devices=8)
    xT_d = nc.dram_tensor("xT", [C, nseq], bf16, kind="ExternalInput")
    wq_d = nc.dram_tensor("wqkvT", [C, 6 * P], bf16, kind="ExternalInput")
    wp_d = nc.dram_tensor("wprojT", [P, 2, C], bf16, kind="ExternalInput")
    id_d = nc.dram_tensor("ident", [P, P], bf16, kind="ExternalInput")
    out_d = nc.dram_tensor("out", [nseq, C], bf16, kind="ExternalOutput")

    cp_state = [0]

    def cp(out, in_):
        cp_state[0] ^= 1
        if cp_state[0]:
            nc.vector.tensor_copy(out, in_)
        else:
            nc.scalar.copy(out, in_)

    with tile.TileContext(nc) as tc, ExitStack() as ctx:
        persist = ctx.enter_context(tc.tile_pool(name="persist", bufs=1))
        qkpool = ctx.enter_context(tc.tile_pool(name="qkpool", bufs=1))
        v1pool = ctx.enter_context(tc.tile_pool(name="v1pool", bufs=1))
        etpools = {}
        for key, bufs in ET_BUFS.items():
            etpools[key] = ctx.enter_context(
                tc.tile_pool(name=f"et{key[0]}{key[1]}", bufs=bufs)
            )

        ones_f32 = persist.tile([P, 1], f32)
        nc.vector.memset(ones_f32, 1.0)

        # q^T/k^T in bf16, packed per PAIR: slot p = q of pair p, slot 2+p
        # = k of pair p; unit 2p at partitions 0:64, unit 2p+1 at 64:128.
        qk_sb = qkpool.tile([P, 4, nseq], bf16)
        # v' natural [j_part, u, jt, 68]: cols 0:64 v (bf16), col 64 ones.
        v1 = v1pool.tile([P, 4, NJT, 68], bf16)
        nc.vector.memset(v1[:, :, :, HD : HD + 1], 1.0)
        # f32r twin of v' for JIT (f32r) E tiles
        v1f = v1pool.tile([P, 4, NJT, 68], f32r)

        nc.vector.tensor_copy(
            v1f[:, :, :, HD : HD + 1],
            ones_f32[:, None, None, :].to_broadcast([P, 4, NJT, 1]),
        )

        # prime the ACT exp table early
        prime = persist.tile([P, 1], f32)
        nc.scalar.activation(prime, ones_f32, EXP, scale=0.0)

        # ---- S-tile scheduler state ----
        # pool (p, h): tile k -> (iq = 2h + k // NJT, jt = k % NJT),
        # iq-major == O consumption order (q2-outer passes).
        et_tiles = {}
        cursor = {ph: 0 for ph in POOL_ORDER}
        consumed = {ph: 0 for ph in POOL_ORDER}
        psS_all = ctx.enter_context(tc.tile_pool(name="psSall", bufs=2, space="PSUM"))
        psS_pool_ref = [psS_all]

        etf_pools_ref = [None]

        def emit_stile(p, h, k, jit=False):
            iq, jt = 2 * h + k // NJT, k % NJT
            psS = psS_pool_ref[0].tile([P, 1024], f32, tag="psS")
            if jit:
                et = etf_pools_ref[0][(p, h)].tile(
                    [P, 1024], f32r, tag=f"etf{p}{h}", name=f"etf_{p}_{jt}_{iq}"
                )
            else:
                et = etpools[(p, h)].tile(
                    [P, 1024], bf16, tag=f"et{p}{h}", name=f"et_{p}_{jt}_{iq}"
                )
            for half in range(2):
                u = 2 * p + half
                pb = 64 * half
                nc.tensor.matmul(
                    psS[:, half * 512 : half * 512 + 512],
                    lhsT=qk_sb[pb : pb + 64, 2 + p, jt * P : (jt + 1) * P],
                    rhs=qk_sb[pb : pb + 64, p, iq * 512 : (iq + 1) * 512],
                    start=True,
                    stop=True,
                )
            nc.scalar.activation(et, psS, EXP, scale=SCALE)
            et_tiles[(p, jt, iq)] = (et, jit)
            cursor[(p, h)] = k + 1

        def try_weave(budget, avail):
            n = 0
            for ph in POOL_ORDER:
                p, h = ph
                while n < budget and cursor[ph] < 2 * NJT:
                    k = cursor[ph]
                    if cursor[ph] - consumed[ph] >= ET_BUFS[ph]:
                        break
                    if not avail(p, h, k):
                        break
                    emit_stile(p, h, k)
                    n += 1
            return n

        # ======== scope A: qkv matmuls, v' build, early S/exp ========
        with (
            tc.tile_pool(name="xtc", bufs=2) as xtc,
            tc.tile_pool(name="vtc", bufs=2) as vtc,
            tc.tile_pool(name="scopeA", bufs=1) as scopeA,
            tc.tile_pool(name="psQ", bufs=3, space="PSUM") as psQ,
            tc.tile_pool(name="psAv", bufs=1, space="PSUM") as psAv,
        ):
            identR = scopeA.tile([P, P], bf16)
            nc.sync.dma_start(identR, id_d.ap())
            wq_sb = scopeA.tile([P, 8, 6 * P], bf16)
            wq_r = wq_d.ap().rearrange("(co p) d -> p co d", p=P)
            xT_tiles = {}

            ld_state = [0]

            def load_chunk(nch, eng):
                ldq = [nc.sync, nc.scalar]
                for co in range(8):
                    xt_c = xtc.tile(
                        [P, QCH], bf16, tag=f"xtc{co}", name=f"xT_{nch}_{co}"
                    )
                    xT_tiles[(nch, co)] = xt_c
                    if eng is None:
                        e = ldq[ld_state[0] % 2]
                        ld_state[0] += 1
                    else:
                        e = eng
                    e.dma_start(
                        xt_c,
                        xT_d[co * P : (co + 1) * P, nch * QCH : (nch + 1) * QCH],
                    )

            # startup: wq + chunk0 on the two hwdge queues (gpsimd DMA
            # enqueue is ~770ns each and would serialize the start)
            ldq = [nc.sync, nc.scalar]
            for co in range(8):
                ldq[co % 2].dma_start(wq_sb[:, co, :], wq_r[:, co, :])
            load_chunk(0, None)
            wp_sb = persist.tile([P, 2, C], bf16)

            done_chunks = [0]

            def avail_early(p, h, k):
                iq, jt = 2 * h + k // NJT, k % NJT
                return jt < 4 * done_chunks[0] and iq < done_chunks[0]

            qk_done = done_chunks

            vT_tiles = {}

            def v_group(nch, u):
                vT = vT_tiles[nch]
                pb = 64 * (u % 2)
                vT_u = vT[pb : pb + 64, u // 2, :]
                ps = psAv.tile([P, 4, HD], bf16, tag="psAv")
                for k in range(4):
                    nc.tensor.transpose(
                        ps[:, k, :],
                        vT_u[:, k * P : (k + 1) * P],
                        identR[pb : pb + 64, pb : pb + 64],
                    )
                nc.vector.tensor_copy(v1[:, u, nch * 4 : (nch + 1) * 4, 0:HD], ps)
                nc.vector.tensor_copy(v1f[:, u, nch * 4 : (nch + 1) * 4, 0:HD], ps)

            for nch in range(NCH):
                if nch + 1 < NCH:
                    load_chunk(nch + 1, None)
                if nch == 1:
                    nc.gpsimd.dma_start(wp_sb, wp_d.ap())
                vT_tiles[nch] = vtc.tile([P, 2, QCH], bf16, tag="vtc", name=f"vT_{nch}")
                vT = vT_tiles[nch]
                sl = slice(nch * QCH, (nch + 1) * QCH)
                for mt in range(6):
                    # two INDEPENDENT half-contraction accumulators in
                    # different PSUM banks; K=64 MMs alternate row-halves so
                    # consecutive MMs hit disjoint row-groups and different
                    # banks (LDW pull-ahead + dual-stream, the S-pair
                    # pattern). Halves are summed by the drain add.
                    psd = [
                        psQ.tile([P, QCH], f32, tag="psQ", name=f"psQ_{nch}_{mt}_{kh}")
                        for kh in range(2)
                    ]
                    for co in range(8):
                        for kh in range(2):
                            pb = 64 * kh
                            nc.tensor.matmul(
                                psd[kh],
                                lhsT=wq_sb[pb : pb + 64, co, mt * P : (mt + 1) * P],
                                rhs=xT_tiles[(nch, co)][pb : pb + 64, :],
                                start=(co == 0),
                                stop=(co == 7),
                            )
                    tmp = xtc.tile([P, QCH], f32, tag="qtmp", name=f"qtmp_{nch}_{mt}")
                    nc.vector.tensor_copy(tmp, psd[1])
                    if mt < 4:
                        # mt 0,1 -> q pair 0,1 ; mt 2,3 -> k pair 0,1
                        nc.vector.tensor_add(qk_sb[:, mt, sl], psd[0], tmp)
                    else:
                        nc.vector.tensor_add(vT[:, mt - 4, :], psd[0], tmp)
                    if nch >= 1 and mt < 4:
                        v_group(nch - 1, mt)
                    if mt == 3:
                        done_chunks[0] = nch + 1
                    try_weave(3 if nch >= 2 else 2, avail_early)
                for co in range(8):
                    del xT_tiles[(nch, co)]
            for u in range(4):
                v_group(NCH - 1, u)
            try_weave(8, avail_early)

        # ======== attention ========
        avail_all = lambda p, h, k: True
        with tc.tile_pool(name="otpool", bufs=1) as otpool:
            OT = otpool.tile([P, 2, nseq], bf16)

            with (
                tc.tile_pool(name="small", bufs=2) as small,
                tc.tile_pool(name="etf00", bufs=2) as etf00,
                tc.tile_pool(name="etf01", bufs=2) as etf01,
                tc.tile_pool(name="etf10", bufs=2) as etf10,
                tc.tile_pool(name="etf11", bufs=2) as etf11,
                tc.tile_pool(name="psO", bufs=4, space="PSUM") as psO,
            ):
                etf_pools_ref[0] = {
                    (0, 0): etf00, (0, 1): etf01, (1, 0): etf10, (1, 1): etf11
                }

                for p in range(2):
                    for h in range(2):
                        ph = (p, h)
                        for q2 in range(2):
                            iq = 2 * h + q2
                            psO_u = [
                                psO.tile(
                                    [P, 512], f32, tag="psO", name=f"psO_{p}{h}{q2}{u}"
                                )
                                for u in range(2)
                            ]
                            for jt in range(NJT):
                                while cursor[ph] < q2 * NJT + jt + 1:
                                    emit_stile(p, h, cursor[ph], jit=True)
                                try_weave(2, avail_all)
                                for half in range(2):
                                    u = 2 * p + half
                                    et, isjit = et_tiles[(p, jt, iq)]
                                    vv = v1f if isjit else v1
                                    nc.tensor.matmul(
                                        psO_u[half][0:65, :],
                                        lhsT=vv[:, u, jt, 0:65],
                                        rhs=et[:, half * 512 : half * 512 + 512],
                                        start=(jt == 0),
                                        stop=(jt == NJT - 1),
                                    )
                                consumed[ph] = q2 * NJT + jt + 1
                            for jt in range(NJT):
                                et_tiles.pop((p, jt, iq), None)
                            # normalize this quarter; psO rows 0:64 = O'^T,
                            # row 64 = rowsum
                            icols = slice(iq * 512, (iq + 1) * 512)
                            last_q = p == 1 and h == 1 and q2 == 1
                            for half in range(2):
                                pb = 64 * half
                                rs = small.tile([1, 512], f32, tag="rs")
                                nc.vector.tensor_copy(
                                    rs, psO_u[half][HD : HD + 1, :]
                                )
                                recip = small.tile([1, 512], f32, tag="recip")
                                nc.vector.reciprocal_approx_fast(recip, rs)
                                bcast = small.tile([64, 512], f32, tag="bcast")
                                nc.gpsimd.partition_broadcast(bcast, recip)
                                if last_q:
                                    # keep the PE HAM-warm through the final
                                    # drain so proj doesn't start throttled
                                    dmy = psO.tile(
                                        [P, 512], f32, tag="psO",
                                        name=f"dmy_{half}",
                                    )
                                    nc.tensor.matmul(
                                        dmy,
                                        lhsT=bcast[0:64, 0:P],
                                        rhs=bcast[0:64, :],
                                        start=True,
                                        stop=True,
                                    )
                                nc.vector.tensor_mul(
                                    OT[pb : pb + 64, p, icols],
                                    psO_u[half][0:64, :],
                                    bcast,
                                )

            # ======== proj ========
            with (
                tc.tile_pool(name="opool", bufs=4) as opool,
                tc.tile_pool(name="psP", bufs=4, space="PSUM") as psP,
            ):
                pdma = [nc.sync, nc.scalar, nc.gpsimd]
                for it in range(NIT):
                    for ech in range(2):
                        ps = psP.tile([P, ECH], f32, tag="psP", name=f"psP_{it}_{ech}")
                        for co in range(2):
                            nc.tensor.matmul(
                                ps,
                                lhsT=OT[:, co, it * P : (it + 1) * P],
                                rhs=wp_sb[:, co, ech * ECH : (ech + 1) * ECH],
                                start=(co == 0),
                                stop=(co == 1),
                            )
                        ot = opool.tile([P, ECH], bf16, tag="opool", name=f"ot_{it}_{ech}")
                        cp(ot, ps)
                        pdma[(2 * it + ech) % 3].dma_start(
                            out_d[it * P : (it + 1) * P, ech * ECH : (ech + 1) * ECH],
                            ot,
                        )

    nc.compile()
    return nc


def get_nc(nseq=NSEQ):
    if nseq not in _cache:
        _cache[nseq] = _build(nseq)
    return _cache[nseq]


def make_in_maps(x, w_qkv, w_proj, nseq=NSEQ):
    import ml_dtypes

    bf = ml_dtypes.bfloat16
    x = np.ascontiguousarray(x, dtype=np.float32)
    w_qkv = np.ascontiguousarray(w_qkv, dtype=np.float32)
    w_proj = np.ascontiguousarray(w_proj, dtype=np.float32)
    in_maps = []
    xT = [np.ascontiguousarray(x[b, :nseq].T.astype(bf)) for b in range(B)]
    ident = np.eye(P, dtype=np.float32).astype(bf)
    for core in range(8):
        b, hg = core // 4, core % 4
        hs = 4 * hg
        wsel = np.empty((6, P, C), np.float32)
        for mt in range(6):
            t, half = mt // 2, mt % 2
            r0 = t * C + (hs + 2 * half) * HD
            wsel[mt] = w_qkv[r0 : r0 + P, :]
        wqkvT = np.ascontiguousarray(
            wsel.transpose(2, 0, 1).reshape(C, 6 * P).astype(bf)
        )
        wp = np.empty((P, 2, C), np.float32)
        for co in range(2):
            c0 = (hs + 2 * co) * HD
            wp[:, co, :] = w_proj[:, c0 : c0 + P].T
        wp = wp.astype(bf)
        in_maps.append(
            {
                "xT": xT[b],
                "wqkvT": wqkvT,
                "wprojT": wp,
                "ident": ident,
            }
        )
    return in_maps


def kernel(x, w_qkv, w_proj, b_proj):
    from concourse.bass_utils import run_bass_kernel_spmd

    nc = get_nc()
    in_maps = make_in_maps(x, w_qkv, w_proj)
    res = run_bass_kernel_spmd(nc, in_maps, core_ids=list(range(8)))
    parts = [np.asarray(r["out"]).astype(np.float32) for r in res.results]
    out = np.stack(
        [
            parts[0] + parts[1] + parts[2] + parts[3],
            parts[4] + parts[5] + parts[6] + parts[7],
        ],
        axis=0,
    )
    return (out + np.asarray(b_proj, np.float32)).astype(np.float32)


# revision 29
# speedup vs baseline: 1.0266x; 1.0266x over previous
# Multi-head attention (B=2, N=2048, C=1024, H=16) on 8 trn2 NeuronCores.
#
# Sharding: core = (batch b = core//4, head-group hg = core%4, 4 heads each).
# Each core computes qkv/attention/proj for its 4 heads of its batch and
# returns a partial projection output [N, C]; the host sums the 4 partials
# per batch and adds b_proj.
#
# Final structure (v7):
#   - x is transposed on the HOST and fed as bf16 xT [C, N]; w_qkv also
#     bf16 -> qkv matmuls are all-bf16 (FWL weight loads, half the DMA).
#   - S^T matmuls contract K=64 with the two heads of a pair packed at
#     partitions 0:64 / 64:128; tile_position row-tiling (auto-inferred)
#     lets the PE run both heads' S matmuls concurrently.
#   - exp runs on ACT only (hard floor ~130us at 1 col/cycle); S tiles are
#     computed as early as dependencies allow (interleaved into the qkv
#     phase) and bf16 E^T tiles hoarded in SBUF (~48 tiles) so ACT is
#     saturated from ~15us in. All PSUM->SBUF drains go to DVE.
#   - E-tile pools are ring buffers in iq-major order; attention passes
#     run q2 (512-wide query quarters) OUTER so each quarter's O
#     accumulator [65, 512] finishes early and its normalize (DVE recip +
#     gpsimd partition-broadcast + multiply from PSUM) hides under the
#     next quarter's matmuls. psO quarters are single banks (bufs=4).
#   - exp is mixed-precision: tiles hoarded during the qkv phase are bf16
#     (slower ACT write, but scope-A ACT has slack and bf16 doubles the
#     hoard capacity); just-in-time tiles during attention are f32r
#     (~1.0us vs ~1.25us per [128,1024] tile). v' is kept in both dtypes.
#   - proj (bf16 OT x bf16 wp) follows immediately; dummy matmuls gated on
#     the last normalize chain keep the PE HAM-warm into proj; output is
#     written bf16 (halves out-DMA) and summed across cores in f32 on host.
import sys

import numpy as np

if "/opt/trn_rl_repo" not in sys.path:
    sys.path.insert(0, "/opt/trn_rl_repo")

B, NSEQ, C = 2, 2048, 1024
H, HD = 16, 64
P = 128
SCALE = HD**-0.5

_cache = {}


def _build(nseq):
    from contextlib import ExitStack

    import concourse.tile as tile
    from concourse import bacc, mybir

    f32 = mybir.dt.float32
    f32r = mybir.dt.float32r
    bf16 = mybir.dt.bfloat16
    EXP = mybir.ActivationFunctionType.Exp

    NJT = nseq // P            # key tiles (128 each)
    NIT = nseq // P            # query tiles for proj
    QCH = 512                  # qkv seq chunk
    NCH = nseq // QCH          # 4
    ECH = 512                  # proj output chunk

    # E^T tile pools (bf16 [128, 1024] = 2KB/partition each, one per
    # (pair, jt, iq) covering both units), keyed (pair, ihalf).
    ET_BUFS = {(0, 0): 24, (0, 1): 6, (1, 0): 12, (1, 1): 6}
    POOL_ORDER = [(0, 0), (0, 1), (1, 0), (1, 1)]

    nc = bacc.Bacc("TRN2", target_bir_lowering=False, debug=False, num_devices=8)
    xT_d = nc.dram_tensor("xT", [C, nseq], bf16, kind="ExternalInput")
    wq_d = nc.dram_tensor("wqkvT", [C, 6 * P], bf16, kind="ExternalInput")
    wp_d = nc.dram_tensor("wprojT", [P, 2, C], bf16, kind="ExternalInput")
    id_d = nc.dram_tensor("ident", [P, P], bf16, kind="ExternalInput")
    out_d = nc.dram_tensor("out", [nseq, C], bf16, kind="ExternalOutput")

    cp_state = [0]

    def cp(out, in_):
        cp_state[0] ^= 1
        if cp_state[0]:
            nc.vector.tensor_copy(out, in_)
        else:
            nc.scalar.copy(out, in_)

    with tile.TileContext(nc) as tc, ExitStack() as ctx:
        persist = ctx.enter_context(tc.tile_pool(name="persist", bufs=1))
        qkpool = ctx.enter_context(tc.tile_pool(name="qkpool", bufs=1))
        v1pool = ctx.enter_context(tc.tile_pool(name="v1pool", bufs=1))
        etpools = {}
        for key, bufs in ET_BUFS.items():
            etpools[key] = ctx.enter_context(
                tc.tile_pool(name=f"et{key[0]}{key[1]}", bufs=bufs)
            )

        ones_f32 = persist.tile([P, 1], f32)
        nc.vector.memset(ones_f32, 1.0)

        # q^T/k^T in bf16, packed per PAIR: slot p = q of pair p, slot 2+p
        # = k of pair p; unit 2p at partitions 0:64, unit 2p+1 at 64:128.
        qk_sb = qkpool.tile([P, 4, nseq], bf16)
        # v' natural [j_part, u, jt, 68]: cols 0:64 v (bf16), col 64 ones.
        v1 = v1pool.tile([P, 4, NJT, 68], bf16)
        nc.vector.memset(v1[:, :, :, HD : HD + 1], 1.0)
        # f32r twin of v' for JIT (f32r) E tiles
        v1f = v1pool.tile([P, 4, NJT, 68], f32r)

        nc.vector.tensor_copy(
            v1f[:, :, :, HD : HD + 1],
            ones_f32[:, None, None, :].to_broadcast([P, 4, NJT, 1]),
        )

        # prime the ACT exp table early
        prime = persist.tile([P, 1], f32)
        nc.scalar.activation(prime, ones_f32, EXP, scale=0.0)

        # ---- S-tile scheduler state ----
        # pool (p, h): tile k -> (iq = 2h + k // NJT, jt = k % NJT),
        # iq-major == O consumption order (q2-outer passes).
        et_tiles = {}
        cursor = {ph: 0 for ph in POOL_ORDER}
        consumed = {ph: 0 for ph in POOL_ORDER}
        psS_all = ctx.enter_context(tc.tile_pool(name="psSall", bufs=2, space="PSUM"))
        psS_pool_ref = [psS_all]

        etf_pools_ref = [None]

        def emit_stile(p, h, k, jit=False):
            iq, jt = 2 * h + k // NJT, k % NJT
            psS = psS_pool_ref[0].tile([P, 1024], f32, tag="psS")
            if jit:
                et = etf_pools_ref[0][(p, h)].tile(
                    [P, 1024], f32r, tag=f"etf{p}{h}", name=f"etf_{p}_{jt}_{iq}"
                )
            else:
                et = etpools[(p, h)].tile(
                    [P, 1024], bf16, tag=f"et{p}{h}", name=f"et_{p}_{jt}_{iq}"
                )
            for half in range(2):
                u = 2 * p + half
                pb = 64 * half
                nc.tensor.matmul(
                    psS[:, half * 512 : half * 512 + 512],
                    lhsT=qk_sb[pb : pb + 64, 2 + p, jt * P : (jt + 1) * P],
                    rhs=qk_sb[pb : pb + 64, p, iq * 512 : (iq + 1) * 512],
                    start=True,
                    stop=True,
                )
            nc.scalar.activation(et, psS, EXP, scale=SCALE)
            et_tiles[(p, jt, iq)] = (et, jit)
            cursor[(p, h)] = k + 1

        def try_weave(budget, avail):
            n = 0
            for ph in POOL_ORDER:
                p, h = ph
                while n < budget and cursor[ph] < 2 * NJT:
                    k = cursor[ph]
                    if cursor[ph] - consumed[ph] >= ET_BUFS[ph]:
                        break
                    if not avail(p, h, k):
                        break
                    emit_stile(p, h, k)
                    n += 1
            return n

        # ======== scope A: qkv matmuls, v' build, early S/exp ========
        with (
            tc.tile_pool(name="xtc", bufs=2) as xtc,
            tc.tile_pool(name="vtc", bufs=2) as vtc,
            tc.tile_pool(name="scopeA", bufs=1) as scopeA,
            tc.tile_pool(name="psQ", bufs=3, space="PSUM") as psQ,
            tc.tile_pool(name="psAv", bufs=1, space="PSUM") as psAv,
        ):
            identR = scopeA.tile([P, P], bf16)
            nc.sync.dma_start(identR, id_d.ap())
            wq_sb = scopeA.tile([P, 8, 6 * P], bf16)
            wq_r = wq_d.ap().rearrange("(co p) d -> p co d", p=P)
            xT_tiles = {}

            ld_state = [0]

            def load_chunk(nch, eng):
                ldq = [nc.sync, nc.scalar]
                for co in range(8):
                    xt_c = xtc.tile(
                        [P, QCH], bf16, tag=f"xtc{co}", name=f"xT_{nch}_{co}"
                    )
                    xT_tiles[(nch, co)] = xt_c
                    if eng is None:
                        e = ldq[ld_state[0] % 2]
                        ld_state[0] += 1
                    else:
                        e = eng
                    e.dma_start(
                        xt_c,
                        xT_d[co * P : (co + 1) * P, nch * QCH : (nch + 1) * QCH],
                    )

            # startup: wq + chunk0 on the two hwdge queues (gpsimd DMA
            # enqueue is ~770ns each and would serialize the start)
            ldq = [nc.sync, nc.scalar]
            for co in range(8):
                ldq[co % 2].dma_start(wq_sb[:, co, :], wq_r[:, co, :])
            load_chunk(0, None)
            wp_sb = persist.tile([P, 2, C], bf16)

            done_chunks = [0]

            def avail_early(p, h, k):
                iq, jt = 2 * h + k // NJT, k % NJT
                return jt < 4 * done_chunks[0] and iq < done_chunks[0]

            qk_done = done_chunks

            vT_tiles = {}

            def v_group(nch, u):
                vT = vT_tiles[nch]
                pb = 64 * (u % 2)
                vT_u = vT[pb : pb + 64, u // 2, :]
                ps = psAv.tile([P, 4, HD], bf16, tag="psAv")
                for k in range(4):
                    nc.tensor.transpose(
                        ps[:, k, :],
                        vT_u[:, k * P : (k + 1) * P],
                        identR[pb : pb + 64, pb : pb + 64],
                    )
                nc.vector.tensor_copy(v1[:, u, nch * 4 : (nch + 1) * 4, 0:HD], ps)
                nc.vector.tensor_copy(v1f[:, u, nch * 4 : (nch + 1) * 4, 0:HD], ps)

            for nch in range(NCH):
                if nch + 1 < NCH:
                    load_chunk(nch + 1, None)
                if nch == 1:
                    nc.gpsimd.dma_start(wp_sb, wp_d.ap())
                vT_tiles[nch] = vtc.tile([P, 2, QCH], bf16, tag="vtc", name=f"vT_{nch}")
                vT = vT_tiles[nch]
                sl = slice(nch * QCH, (nch + 1) * QCH)
                for mt in range(6):
                    # two INDEPENDENT half-contraction accumulators in
                    # different PSUM banks; K=64 MMs alternate row-halves so
                    # consecutive MMs hit disjoint row-groups and different
                    # banks (LDW pull-ahead + dual-stream, the S-pair
                    # pattern). Halves are summed by the drain add.
                    psd = [
                        psQ.tile([P, QCH], f32, tag="psQ", name=f"psQ_{nch}_{mt}_{kh}")
                        for kh in range(2)
                    ]
                    for co in range(8):
                        for kh in range(2):
                            pb = 64 * kh
                            nc.tensor.matmul(
                                psd[kh],
                                lhsT=wq_sb[pb : pb + 64, co, mt * P : (mt + 1) * P],
                                rhs=xT_tiles[(nch, co)][pb : pb + 64, :],
                                start=(co == 0),
                                stop=(co == 7),
                            )
                    tmp = xtc.tile([P, QCH], f32, tag="qtmp", name=f"qtmp_{nch}_{mt}")
                    nc.vector.tensor_copy(tmp, psd[1])
                    if mt < 4:
                        # mt 0,1 -> q pair 0,1 ; mt 2,3 -> k pair 0,1
                        nc.vector.tensor_add(qk_sb[:, mt, sl], psd[0], tmp)
                    else:
                        nc.vector.tensor_add(vT[:, mt - 4, :], psd[0], tmp)
                    if nch >= 1 and mt < 4:
                        v_group(nch - 1, mt)
                    if mt == 3:
                        done_chunks[0] = nch + 1
                    try_weave(2, avail_early)
                for co in range(8):
                    del xT_tiles[(nch, co)]
            for u in range(4):
                v_group(NCH - 1, u)
            try_weave(4, avail_early)

        # ======== attention ========
        avail_all = lambda p, h, k: True
        with tc.tile_pool(name="otpool", bufs=1) as otpool:
            OT = otpool.tile([P, 2, nseq], bf16)

            with (
                tc.tile_pool(name="small", bufs=2) as small,
                tc.tile_pool(name="etf00", bufs=2) as etf00,
                tc.tile_pool(name="etf01", bufs=2) as etf01,
                tc.tile_pool(name="etf10", bufs=2) as etf10,
                tc.tile_pool(name="etf11", bufs=2) as etf11,
                tc.tile_pool(name="psO", bufs=4, space="PSUM") as psO,
            ):
                etf_pools_ref[0] = {
                    (0, 0): etf00, (0, 1): etf01, (1, 0): etf10, (1, 1): etf11
                }

                for p in range(2):
                    for h in range(2):
                        ph = (p, h)
                        for q2 in range(2):
                            iq = 2 * h + q2
                            psO_u = [
                                psO.tile(
                                    [P, 512], f32, tag="psO", name=f"psO_{p}{h}{q2}{u}"
                                )
                                for u in range(2)
                            ]
                            for jt in range(NJT):
                                while cursor[ph] < q2 * NJT + jt + 1:
                                    emit_stile(p, h, cursor[ph], jit=True)
                                try_weave(2, avail_all)
                                for half in range(2):
                                    u = 2 * p + half
                                    et, isjit = et_tiles[(p, jt, iq)]
                                    vv = v1f if isjit else v1
                                    nc.tensor.matmul(
                                        psO_u[half][0:65, :],
                                        lhsT=vv[:, u, jt, 0:65],
                                        rhs=et[:, half * 512 : half * 512 + 512],
                                        start=(jt == 0),
                                        stop=(jt == NJT - 1),
                                    )
                                consumed[ph] = q2 * NJT + jt + 1
                            for jt in range(NJT):
                                et_tiles.pop((p, jt, iq), None)
                            # normalize this quarter; psO rows 0:64 = O'^T,
                            # row 64 = rowsum
                            icols = slice(iq * 512, (iq + 1) * 512)
                            last_q = p == 1 and h == 1 and q2 == 1
                            for half in range(2):
                                pb = 64 * half
                                rs = small.tile([1, 512], f32, tag="rs")
                                nc.vector.tensor_copy(
                                    rs, psO_u[half][HD : HD + 1, :]
                                )
                                recip = small.tile([1, 512], f32, tag="recip")
                                nc.vector.reciprocal_approx_fast(recip, rs)
                                bcast = small.tile([64, 512], f32, tag="bcast")
                                nc.gpsimd.partition_broadcast(bcast, recip)
                                if last_q:
                                    # keep the PE HAM-warm through the final
                                    # drain so proj doesn't start throttled
                                    dmy = psO.tile(
                                        [P, 512], f32, tag="psO",
                                        name=f"dmy_{half}",
                                    )
                                    nc.tensor.matmul(
                                        dmy,
                                        lhsT=bcast[0:64, 0:P],
                                        rhs=bcast[0:64, :],
                                        start=True,
                                        stop=True,
                                    )
                                nc.vector.tensor_mul(
                                    OT[pb : pb + 64, p, icols],
                                    psO_u[half][0:64, :],
                                    bcast,
                                )

            # ======== proj ========
            with (
                tc.tile_pool(name="opool", bufs=4) as opool,
                tc.tile_pool(name="psP", bufs=4, space="PSUM") as psP,
            ):
                pdma = [nc.sync, nc.scalar, nc.gpsimd]
                for it in range(NIT):
                    for ech in range(2):
                        ps = psP.tile([P, ECH], f32, tag="psP", name=f"psP_{it}_{ech}")
                        for co in range(2):
                            nc.tensor.matmul(
                                ps,
                                lhsT=OT[:, co, it * P : (it + 1) * P],
                                rhs=wp_sb[:, co, ech * ECH : (ech + 1) * ECH],
                                start=(co == 0),
                                stop=(co == 1),
                            )
                        ot = opool.tile([P, ECH], bf16, tag="opool", name=f"ot_{it}_{ech}")
                        cp(ot, ps)
                        pdma[(2 * it + ech) % 3].dma_start(
                            out_d[it * P : (it + 1) * P, ech * ECH : (ech + 1) * ECH],
                            ot,
                        )

    nc.compile()
    return nc


def get_nc(nseq=NSEQ):
    if nseq not in _cache:
        _cache[nseq] = _build(nseq)
    return _cache[nseq]


def make_in_maps(x, w_qkv, w_proj, nseq=NSEQ):
    import ml_dtypes

    bf = ml_dtypes.bfloat16
    x = np.ascontiguousarray(x, dtype=np.float32)
    w_qkv = np.ascontiguousarray(w_qkv, dtype=np.float32)
    w_proj = np.ascontiguousarray(w_proj, dtype=np.float32)
    in_maps = []
    xT = [np.ascontiguousarray(x[b, :nseq].T.astype(bf)) for b in range(B)]
    ident = np.eye(P, dtype=np.float32).astype(bf)
    for core in range(8):
        b, hg = core // 4, core % 4
        hs = 4 * hg
        wsel = np.empty((6, P, C), np.float32)
        for mt in range(6):
            t, half = mt // 2, mt % 2
            r0 = t * C + (hs + 2 * half) * HD
            wsel[mt] = w_qkv[r0 : r0 + P, :]
        wqkvT = np.ascontiguousarray(
            wsel.transpose(2, 0, 1).reshape(C, 6 * P).astype(bf)
        )
        wp = np.empty((P, 2, C), np.float32)
        for co in range(2):
            c0 = (hs + 2 * co) * HD
            wp[:, co, :] = w_proj[:, c0 : c0 + P].T
        wp = wp.astype(bf)
        in_maps.append(
            {
                "xT": xT[b],
                "wqkvT": wqkvT,
                "wprojT": wp,
                "ident": ident,
            }
        )
    return in_maps


def kernel(x, w_qkv, w_proj, b_proj):
    from concourse.bass_utils import run_bass_kernel_spmd

    nc = get_nc()
    in_maps = make_in_maps(x, w_qkv, w_proj)
    res = run_bass_kernel_spmd(nc, in_maps, core_ids=list(range(8)))
    parts = [np.asarray(r["out"]).astype(np.float32) for r in res.results]
    out = np.stack(
        [
            parts[0] + parts[1] + parts[2] + parts[3],
            parts[4] + parts[5] + parts[6] + parts[7],
        ],
        axis=0,
    )
    return (out + np.asarray(b_proj, np.float32)).astype(np.float32)


# revision 30
# speedup vs baseline: 1.0346x; 1.0079x over previous
# Multi-head attention (B=2, N=2048, C=1024, H=16) on 8 trn2 NeuronCores.
#
# Sharding: core = (batch b = core//4, head-group hg = core%4, 4 heads each).
# Each core computes qkv/attention/proj for its 4 heads of its batch and
# returns a partial projection output [N, C]; the host sums the 4 partials
# per batch and adds b_proj.
#
# Final structure (v7):
#   - x is transposed on the HOST and fed as bf16 xT [C, N]; w_qkv also
#     bf16 -> qkv matmuls are all-bf16 (FWL weight loads, half the DMA).
#   - S^T matmuls contract K=64 with the two heads of a pair packed at
#     partitions 0:64 / 64:128; tile_position row-tiling (auto-inferred)
#     lets the PE run both heads' S matmuls concurrently.
#   - exp runs on ACT only (hard floor ~130us at 1 col/cycle); S tiles are
#     computed as early as dependencies allow (interleaved into the qkv
#     phase) and bf16 E^T tiles hoarded in SBUF (~48 tiles) so ACT is
#     saturated from ~15us in. All PSUM->SBUF drains go to DVE.
#   - E-tile pools are ring buffers in iq-major order; attention passes
#     run q2 (512-wide query quarters) OUTER so each quarter's O
#     accumulator [65, 512] finishes early and its normalize (DVE recip +
#     gpsimd partition-broadcast + multiply from PSUM) hides under the
#     next quarter's matmuls. psO quarters are single banks (bufs=4).
#   - exp is mixed-precision: tiles hoarded during the qkv phase are bf16
#     (slower ACT write, but scope-A ACT has slack and bf16 doubles the
#     hoard capacity); just-in-time tiles during attention are f32r
#     (~1.0us vs ~1.25us per [128,1024] tile). v' is kept in both dtypes.
#   - proj (bf16 OT x bf16 wp) follows immediately; dummy matmuls gated on
#     the last normalize chain keep the PE HAM-warm into proj; output is
#     written bf16 (halves out-DMA) and summed across cores in f32 on host.
import sys

import numpy as np

if "/opt/trn_rl_repo" not in sys.path:
    sys.path.insert(0, "/opt/trn_rl_repo")

B, NSEQ, C = 2, 2048, 1024
H, HD = 16, 64
P = 128
SCALE = HD**-0.5

_cache = {}


def _build(nseq):
    from contextlib import ExitStack

    import concourse.tile as tile
    from concourse import bacc, mybir

    f32 = mybir.dt.float32
    f32r = mybir.dt.float32r
    bf16 = mybir.dt.bfloat16
    EXP = mybir.ActivationFunctionType.Exp

    NJT = nseq // P            # key tiles (128 each)
    NIT = nseq // P            # query tiles for proj
    QCH = 512                  # qkv seq chunk
    NCH = nseq // QCH          # 4
    ECH = 512                  # proj output chunk

    # E^T tile pools (bf16 [128, 1024] = 2KB/partition each, one per
    # (pair, jt, iq) covering both units), keyed (pair, ihalf).
    ET_BUFS = {(0, 0): 24, (0, 1): 6, (1, 0): 12, (1, 1): 6}
    POOL_ORDER = [(0, 0), (0, 1), (1, 0), (1, 1)]

    nc = bacc.Bacc("TRN2", target_bir_lowering=False, debug=False, num_devices=8)
    xT_d = nc.dram_tensor("xT", [C, nseq], bf16, kind="ExternalInput")
    wq_d = nc.dram_tensor("wqkvT", [C, 6 * P], bf16, kind="ExternalInput")
    wp_d = nc.dram_tensor("wprojT", [P, 2, C], bf16, kind="ExternalInput")
    id_d = nc.dram_tensor("ident", [P, P], bf16, kind="ExternalInput")
    out_d = nc.dram_tensor("out", [nseq, C], bf16, kind="ExternalOutput")

    cp_state = [0]

    def cp(out, in_):
        cp_state[0] ^= 1
        if cp_state[0]:
            nc.vector.tensor_copy(out, in_)
        else:
            nc.scalar.copy(out, in_)

    with tile.TileContext(nc) as tc, ExitStack() as ctx:
        persist = ctx.enter_context(tc.tile_pool(name="persist", bufs=1))
        qkpool = ctx.enter_context(tc.tile_pool(name="qkpool", bufs=1))
        v1pool = ctx.enter_context(tc.tile_pool(name="v1pool", bufs=1))
        etpools = {}
        for key, bufs in ET_BUFS.items():
            etpools[key] = ctx.enter_context(
                tc.tile_pool(name=f"et{key[0]}{key[1]}", bufs=bufs)
            )

        ones_f32 = persist.tile([P, 1], f32)
        nc.vector.memset(ones_f32, 1.0)

        # q^T/k^T in bf16, packed per PAIR: slot p = q of pair p, slot 2+p
        # = k of pair p; unit 2p at partitions 0:64, unit 2p+1 at 64:128.
        qk_sb = qkpool.tile([P, 4, nseq], bf16)
        # v' natural [j_part, u, jt, 68]: cols 0:64 v (bf16), col 64 ones.
        v1 = v1pool.tile([P, 4, NJT, 68], bf16)
        nc.vector.memset(v1[:, :, :, HD : HD + 1], 1.0)
        # f32r twin of v' for JIT (f32r) E tiles
        v1f = v1pool.tile([P, 4, NJT, 68], f32r)

        nc.vector.tensor_copy(
            v1f[:, :, :, HD : HD + 1],
            ones_f32[:, None, None, :].to_broadcast([P, 4, NJT, 1]),
        )

        # prime the ACT exp table early
        prime = persist.tile([P, 1], f32)
        nc.scalar.activation(prime, ones_f32, EXP, scale=0.0)

        # ---- S-tile scheduler state ----
        # pool (p, h): tile k -> (iq = 2h + k // NJT, jt = k % NJT),
        # iq-major == O consumption order (q2-outer passes).
        et_tiles = {}
        cursor = {ph: 0 for ph in POOL_ORDER}
        consumed = {ph: 0 for ph in POOL_ORDER}
        psS_pool_ref = [None]

        etf_pools_ref = [None]

        def emit_stile(p, h, k, jit=False):
            iq, jt = 2 * h + k // NJT, k % NJT
            psS = psS_pool_ref[0].tile([P, 1024], f32, tag="psS")
            if jit:
                et = etf_pools_ref[0][(p, h)].tile(
                    [P, 1024], f32r, tag=f"etf{p}{h}", name=f"etf_{p}_{jt}_{iq}"
                )
            else:
                et = etpools[(p, h)].tile(
                    [P, 1024], bf16, tag=f"et{p}{h}", name=f"et_{p}_{jt}_{iq}"
                )
            for half in range(2):
                u = 2 * p + half
                pb = 64 * half
                nc.tensor.matmul(
                    psS[:, half * 512 : half * 512 + 512],
                    lhsT=qk_sb[pb : pb + 64, 2 + p, jt * P : (jt + 1) * P],
                    rhs=qk_sb[pb : pb + 64, p, iq * 512 : (iq + 1) * 512],
                    start=True,
                    stop=True,
                )
            nc.scalar.activation(et, psS, EXP, scale=SCALE)
            et_tiles[(p, jt, iq)] = (et, jit)
            cursor[(p, h)] = k + 1

        def try_weave(budget, avail):
            n = 0
            for ph in POOL_ORDER:
                p, h = ph
                while n < budget and cursor[ph] < 2 * NJT:
                    k = cursor[ph]
                    if cursor[ph] - consumed[ph] >= ET_BUFS[ph]:
                        break
                    if not avail(p, h, k):
                        break
                    emit_stile(p, h, k)
                    n += 1
            return n

        # ======== scope A: qkv matmuls, v' build, early S/exp ========
        with (
            tc.tile_pool(name="xtc", bufs=2) as xtc,
            tc.tile_pool(name="vtc", bufs=2) as vtc,
            tc.tile_pool(name="scopeA", bufs=1) as scopeA,
            tc.tile_pool(name="psQ", bufs=3, space="PSUM") as psQ,
            tc.tile_pool(name="psAv", bufs=1, space="PSUM") as psAv,
            tc.tile_pool(name="psSa", bufs=2, space="PSUM") as psSa,
        ):
            psS_pool_ref[0] = psSa
            identR = scopeA.tile([P, P], bf16)
            nc.sync.dma_start(identR, id_d.ap())
            wq_sb = scopeA.tile([P, 8, 6 * P], bf16)
            wq_r = wq_d.ap().rearrange("(co p) d -> p co d", p=P)
            xT_tiles = {}

            ld_state = [0]

            def load_chunk(nch, eng):
                ldq = [nc.sync, nc.scalar]
                for co in range(8):
                    xt_c = xtc.tile(
                        [P, QCH], bf16, tag=f"xtc{co}", name=f"xT_{nch}_{co}"
                    )
                    xT_tiles[(nch, co)] = xt_c
                    if eng is None:
                        e = ldq[ld_state[0] % 2]
                        ld_state[0] += 1
                    else:
                        e = eng
                    e.dma_start(
                        xt_c,
                        xT_d[co * P : (co + 1) * P, nch * QCH : (nch + 1) * QCH],
                    )

            # startup: wq + chunk0 on the two hwdge queues (gpsimd DMA
            # enqueue is ~770ns each and would serialize the start)
            ldq = [nc.sync, nc.scalar]
            for co in range(8):
                ldq[co % 2].dma_start(wq_sb[:, co, :], wq_r[:, co, :])
            load_chunk(0, None)
            wp_sb = persist.tile([P, 2, C], bf16)

            done_chunks = [0]

            def avail_early(p, h, k):
                iq, jt = 2 * h + k // NJT, k % NJT
                return jt < 4 * done_chunks[0] and iq < done_chunks[0]

            qk_done = done_chunks

            vT_tiles = {}

            def v_group(nch, u):
                vT = vT_tiles[nch]
                pb = 64 * (u % 2)
                vT_u = vT[pb : pb + 64, u // 2, :]
                ps = psAv.tile([P, 4, HD], bf16, tag="psAv")
                for k in range(4):
                    nc.tensor.transpose(
                        ps[:, k, :],
                        vT_u[:, k * P : (k + 1) * P],
                        identR[pb : pb + 64, pb : pb + 64],
                    )
                nc.vector.tensor_copy(v1[:, u, nch * 4 : (nch + 1) * 4, 0:HD], ps)
                nc.vector.tensor_copy(v1f[:, u, nch * 4 : (nch + 1) * 4, 0:HD], ps)

            for nch in range(NCH):
                if nch + 1 < NCH:
                    load_chunk(nch + 1, None)
                if nch == 1:
                    nc.gpsimd.dma_start(wp_sb, wp_d.ap())
                vT_tiles[nch] = vtc.tile([P, 2, QCH], bf16, tag="vtc", name=f"vT_{nch}")
                vT = vT_tiles[nch]
                sl = slice(nch * QCH, (nch + 1) * QCH)
                for mt in range(6):
                    # two INDEPENDENT half-contraction accumulators in
                    # different PSUM banks; K=64 MMs alternate row-halves so
                    # consecutive MMs hit disjoint row-groups and different
                    # banks (LDW pull-ahead + dual-stream, the S-pair
                    # pattern). Halves are summed by the drain add.
                    psd = [
                        psQ.tile([P, QCH], f32, tag="psQ", name=f"psQ_{nch}_{mt}_{kh}")
                        for kh in range(2)
                    ]
                    for co in range(8):
                        for kh in range(2):
                            pb = 64 * kh
                            nc.tensor.matmul(
                                psd[kh],
                                lhsT=wq_sb[pb : pb + 64, co, mt * P : (mt + 1) * P],
                                rhs=xT_tiles[(nch, co)][pb : pb + 64, :],
                                start=(co == 0),
                                stop=(co == 7),
                            )
                    tmp = xtc.tile([P, QCH], f32, tag="qtmp", name=f"qtmp_{nch}_{mt}")
                    nc.vector.tensor_copy(tmp, psd[1])
                    if mt < 4:
                        # mt 0,1 -> q pair 0,1 ; mt 2,3 -> k pair 0,1
                        nc.vector.tensor_add(qk_sb[:, mt, sl], psd[0], tmp)
                    else:
                        nc.vector.tensor_add(vT[:, mt - 4, :], psd[0], tmp)
                    if nch >= 1 and mt < 4:
                        v_group(nch - 1, mt)
                    if mt == 3:
                        done_chunks[0] = nch + 1
                    try_weave(2, avail_early)
                for co in range(8):
                    del xT_tiles[(nch, co)]
            for u in range(4):
                v_group(NCH - 1, u)
            try_weave(4, avail_early)

        # ======== attention ========
        avail_all = lambda p, h, k: True
        with tc.tile_pool(name="otpool", bufs=1) as otpool:
            OT = otpool.tile([P, 2, nseq], bf16)

            with (
                tc.tile_pool(name="small", bufs=2) as small,
                tc.tile_pool(name="etf00", bufs=2) as etf00,
                tc.tile_pool(name="etf01", bufs=2) as etf01,
                tc.tile_pool(name="etf10", bufs=2) as etf10,
                tc.tile_pool(name="etf11", bufs=2) as etf11,
                tc.tile_pool(name="psS", bufs=2, space="PSUM") as psS,
                tc.tile_pool(name="psO", bufs=4, space="PSUM") as psO,
            ):
                psS_pool_ref[0] = psS
                etf_pools_ref[0] = {
                    (0, 0): etf00, (0, 1): etf01, (1, 0): etf10, (1, 1): etf11
                }

                for p in range(2):
                    for h in range(2):
                        ph = (p, h)
                        for q2 in range(2):
                            iq = 2 * h + q2
                            psO_u = [
                                psO.tile(
                                    [P, 512], f32, tag="psO", name=f"psO_{p}{h}{q2}{u}"
                                )
                                for u in range(2)
                            ]
                            for jt in range(NJT):
                                while cursor[ph] < q2 * NJT + jt + 1:
                                    emit_stile(p, h, cursor[ph], jit=True)
                                try_weave(2, avail_all)
                                for half in range(2):
                                    u = 2 * p + half
                                    et, isjit = et_tiles[(p, jt, iq)]
                                    vv = v1f if isjit else v1
                                    nc.tensor.matmul(
                                        psO_u[half][0:65, :],
                                        lhsT=vv[:, u, jt, 0:65],
                                        rhs=et[:, half * 512 : half * 512 + 512],
                                        start=(jt == 0),
                                        stop=(jt == NJT - 1),
                                    )
                                consumed[ph] = q2 * NJT + jt + 1
                            for jt in range(NJT):
                                et_tiles.pop((p, jt, iq), None)
                            # normalize this quarter; psO rows 0:64 = O'^T,
                            # row 64 = rowsum
                            icols = slice(iq * 512, (iq + 1) * 512)
                            last_q = p == 1 and h == 1 and q2 == 1
                            for half in range(2):
                                pb = 64 * half
                                rs = small.tile([1, 512], f32, tag="rs")
                                nc.vector.tensor_copy(
                                    rs, psO_u[half][HD : HD + 1, :]
                                )
                                recip = small.tile([1, 512], f32, tag="recip")
                                nc.vector.reciprocal_approx_fast(recip, rs)
                                bcast = small.tile([64, 512], f32, tag="bcast")
                                nc.gpsimd.partition_broadcast(bcast, recip)
                                if last_q:
                                    # keep the PE HAM-warm through the final
                                    # drain so proj doesn't start throttled
                                    dmy = psO.tile(
                                        [P, 512], f32, tag="psO",
                                        name=f"dmy_{half}",
                                    )
                                    nc.tensor.matmul(
                                        dmy,
                                        lhsT=bcast[0:64, 0:P],
                                        rhs=bcast[0:64, :],
                                        start=True,
                                        stop=True,
                                    )
                                nc.vector.tensor_mul(
                                    OT[pb : pb + 64, p, icols],
                                    psO_u[half][0:64, :],
                                    bcast,
                                )

            # ======== proj ========
            with (
                tc.tile_pool(name="opool", bufs=4) as opool,
                tc.tile_pool(name="psP", bufs=4, space="PSUM") as psP,
            ):
                pdma = [nc.sync, nc.scalar, nc.gpsimd]
                for it in range(NIT):
                    for ech in range(2):
                        ps = psP.tile([P, ECH], f32, tag="psP", name=f"psP_{it}_{ech}")
                        for co in range(2):
                            nc.tensor.matmul(
                                ps,
                                lhsT=OT[:, co, it * P : (it + 1) * P],
                                rhs=wp_sb[:, co, ech * ECH : (ech + 1) * ECH],
                                start=(co == 0),
                                stop=(co == 1),
                            )
                        ot = opool.tile([P, ECH], bf16, tag="opool", name=f"ot_{it}_{ech}")
                        cp(ot, ps)
                        pdma[(2 * it + ech) % 3].dma_start(
                            out_d[it * P : (it + 1) * P, ech * ECH : (ech + 1) * ECH],
                            ot,
                        )

    nc.compile()
    return nc


def get_nc(nseq=NSEQ):
    if nseq not in _cache:
        _cache[nseq] = _build(nseq)
    return _cache[nseq]


def make_in_maps(x, w_qkv, w_proj, nseq=NSEQ):
    import ml_dtypes

    bf = ml_dtypes.bfloat16
    x = np.ascontiguousarray(x, dtype=np.float32)
    w_qkv = np.ascontiguousarray(w_qkv, dtype=np.float32)
    w_proj = np.ascontiguousarray(w_proj, dtype=np.float32)
    in_maps = []
    xT = [np.ascontiguousarray(x[b, :nseq].T.astype(bf)) for b in range(B)]
    ident = np.eye(P, dtype=np.float32).astype(bf)
    for core in range(8):
        b, hg = core // 4, core % 4
        hs = 4 * hg
        wsel = np.empty((6, P, C), np.float32)
        for mt in range(6):
            t, half = mt // 2, mt % 2
            r0 = t * C + (hs + 2 * half) * HD
            wsel[mt] = w_qkv[r0 : r0 + P, :]
        wqkvT = np.ascontiguousarray(
            wsel.transpose(2, 0, 1).reshape(C, 6 * P).astype(bf)
        )
        wp = np.empty((P, 2, C), np.float32)
        for co in range(2):
            c0 = (hs + 2 * co) * HD
            wp[:, co, :] = w_proj[:, c0 : c0 + P].T
        wp = wp.astype(bf)
        in_maps.append(
            {
                "xT": xT[b],
                "wqkvT": wqkvT,
                "wprojT": wp,
                "ident": ident,
            }
        )
    return in_maps


def kernel(x, w_qkv, w_proj, b_proj):
    from concourse.bass_utils import run_bass_kernel_spmd

    nc = get_nc()
    in_maps = make_in_maps(x, w_qkv, w_proj)
    res = run_bass_kernel_spmd(nc, in_maps, core_ids=list(range(8)))
    parts = [np.asarray(r["out"]).astype(np.float32) for r in res.results]
    out = np.stack(
        [
            parts[0] + parts[1] + parts[2] + parts[3],
            parts[4] + parts[5] + parts[6] + parts[7],
        ],
        axis=0,
    )
    return (out + np.asarray(b_proj, np.float32)).astype(np.float32)


# revision 31
# speedup vs baseline: 1.0517x; 1.0165x over previous
# Multi-head attention (B=2, N=2048, C=1024, H=16) on 8 trn2 NeuronCores.
#
# Sharding: core = (batch b = core//4, head-group hg = core%4, 4 heads each).
# Each core computes qkv/attention/proj for its 4 heads of its batch and
# returns a partial projection output [N, C]; the host sums the 4 partials
# per batch and adds b_proj.
#
# Final structure (v7):
#   - x is transposed on the HOST and fed as bf16 xT [C, N]; w_qkv also
#     bf16 -> qkv matmuls are all-bf16 (FWL weight loads, half the DMA).
#   - S^T matmuls contract K=64 with the two heads of a pair packed at
#     partitions 0:64 / 64:128; tile_position row-tiling (auto-inferred)
#     lets the PE run both heads' S matmuls concurrently.
#   - exp runs on ACT only (hard floor ~130us at 1 col/cycle); S tiles are
#     computed as early as dependencies allow (interleaved into the qkv
#     phase) and bf16 E^T tiles hoarded in SBUF (~48 tiles) so ACT is
#     saturated from ~15us in. All PSUM->SBUF drains go to DVE.
#   - E-tile pools are ring buffers in iq-major order; attention passes
#     run q2 (512-wide query quarters) OUTER so each quarter's O
#     accumulator [65, 512] finishes early and its normalize (DVE recip +
#     gpsimd partition-broadcast + multiply from PSUM) hides under the
#     next quarter's matmuls. psO quarters are single banks (bufs=4).
#   - exp is mixed-precision: tiles hoarded during the qkv phase are bf16
#     (slower ACT write, but scope-A ACT has slack and bf16 doubles the
#     hoard capacity); just-in-time tiles during attention are f32r
#     (~1.0us vs ~1.25us per [128,1024] tile). v' is kept in both dtypes.
#   - proj (bf16 OT x bf16 wp) follows immediately; dummy matmuls gated on
#     the last normalize chain keep the PE HAM-warm into proj; output is
#     written bf16 (halves out-DMA) and summed across cores in f32 on host.
import sys

import numpy as np

if "/opt/trn_rl_repo" not in sys.path:
    sys.path.insert(0, "/opt/trn_rl_repo")

B, NSEQ, C = 2, 2048, 1024
H, HD = 16, 64
P = 128
SCALE = HD**-0.5

_cache = {}


def _build(nseq):
    from contextlib import ExitStack

    import concourse.tile as tile
    from concourse import bacc, mybir

    f32 = mybir.dt.float32
    f32r = mybir.dt.float32r
    bf16 = mybir.dt.bfloat16
    EXP = mybir.ActivationFunctionType.Exp

    NJT = nseq // P            # key tiles (128 each)
    NIT = nseq // P            # query tiles for proj
    QCH = 512                  # qkv seq chunk
    NCH = nseq // QCH          # 4
    ECH = 512                  # proj output chunk

    # E^T tile pools (bf16 [128, 1024] = 2KB/partition each, one per
    # (pair, jt, iq) covering both units), keyed (pair, ihalf).
    ET_BUFS = {(0, 0): 24, (0, 1): 12, (1, 0): 12, (1, 1): 6}
    POOL_ORDER = [(0, 0), (0, 1), (1, 0), (1, 1)]

    nc = bacc.Bacc("TRN2", target_bir_lowering=False, debug=False, num_devices=8)
    xT_d = nc.dram_tensor("xT", [C, nseq], bf16, kind="ExternalInput")
    wq_d = nc.dram_tensor("wqkvT", [C, 6 * P], bf16, kind="ExternalInput")
    wp_d = nc.dram_tensor("wprojT", [P, 2, C], bf16, kind="ExternalInput")
    id_d = nc.dram_tensor("ident", [P, P], bf16, kind="ExternalInput")
    out_d = nc.dram_tensor("out", [nseq, C], bf16, kind="ExternalOutput")

    cp_state = [0]

    def cp(out, in_):
        cp_state[0] ^= 1
        if cp_state[0]:
            nc.vector.tensor_copy(out, in_)
        else:
            nc.scalar.copy(out, in_)

    with tile.TileContext(nc) as tc, ExitStack() as ctx:
        persist = ctx.enter_context(tc.tile_pool(name="persist", bufs=1))
        qkpool = ctx.enter_context(tc.tile_pool(name="qkpool", bufs=1))
        v1pool = ctx.enter_context(tc.tile_pool(name="v1pool", bufs=1))
        etpools = {}
        for key, bufs in ET_BUFS.items():
            etpools[key] = ctx.enter_context(
                tc.tile_pool(name=f"et{key[0]}{key[1]}", bufs=bufs)
            )

        ones_f32 = persist.tile([P, 1], f32)
        nc.vector.memset(ones_f32, 1.0)

        # q^T/k^T in bf16, packed per PAIR: slot p = q of pair p, slot 2+p
        # = k of pair p; unit 2p at partitions 0:64, unit 2p+1 at 64:128.
        qk_sb = qkpool.tile([P, 4, nseq], bf16)
        # v' natural [j_part, u, jt, 68]: cols 0:64 v (bf16), col 64 ones.
        v1 = v1pool.tile([P, 4, NJT, 68], bf16)
        nc.vector.memset(v1[:, :, :, HD : HD + 1], 1.0)
        # f32r twin of v' for JIT (f32r) E tiles
        v1f = v1pool.tile([P, 4, NJT, 68], f32r)

        nc.vector.tensor_copy(
            v1f[:, :, :, HD : HD + 1],
            ones_f32[:, None, None, :].to_broadcast([P, 4, NJT, 1]),
        )

        # prime the ACT exp table early
        prime = persist.tile([P, 1], f32)
        nc.scalar.activation(prime, ones_f32, EXP, scale=0.0)

        # ---- S-tile scheduler state ----
        # pool (p, h): tile k -> (iq = 2h + k // NJT, jt = k % NJT),
        # iq-major == O consumption order (q2-outer passes).
        et_tiles = {}
        cursor = {ph: 0 for ph in POOL_ORDER}
        consumed = {ph: 0 for ph in POOL_ORDER}
        psS_pool_ref = [None]

        etf_pools_ref = [None]

        def emit_stile(p, h, k, jit=False):
            iq, jt = 2 * h + k // NJT, k % NJT
            psS = psS_pool_ref[0].tile([P, 1024], f32, tag="psS")
            if jit:
                et = etf_pools_ref[0][(p, h)].tile(
                    [P, 1024], f32r, tag=f"etf{p}{h}", name=f"etf_{p}_{jt}_{iq}"
                )
            else:
                et = etpools[(p, h)].tile(
                    [P, 1024], bf16, tag=f"et{p}{h}", name=f"et_{p}_{jt}_{iq}"
                )
            for half in range(2):
                u = 2 * p + half
                pb = 64 * half
                nc.tensor.matmul(
                    psS[:, half * 512 : half * 512 + 512],
                    lhsT=qk_sb[pb : pb + 64, 2 + p, jt * P : (jt + 1) * P],
                    rhs=qk_sb[pb : pb + 64, p, iq * 512 : (iq + 1) * 512],
                    start=True,
                    stop=True,
                )
            nc.scalar.activation(et, psS, EXP, scale=SCALE)
            et_tiles[(p, jt, iq)] = (et, jit)
            cursor[(p, h)] = k + 1

        def try_weave(budget, avail):
            n = 0
            for ph in POOL_ORDER:
                p, h = ph
                while n < budget and cursor[ph] < 2 * NJT:
                    k = cursor[ph]
                    if cursor[ph] - consumed[ph] >= ET_BUFS[ph]:
                        break
                    if not avail(p, h, k):
                        break
                    emit_stile(p, h, k)
                    n += 1
            return n

        # ======== scope A: qkv matmuls, v' build, early S/exp ========
        with (
            tc.tile_pool(name="xtc", bufs=2) as xtc,
            tc.tile_pool(name="vtc", bufs=2) as vtc,
            tc.tile_pool(name="scopeA", bufs=1) as scopeA,
            tc.tile_pool(name="psQ", bufs=3, space="PSUM") as psQ,
            tc.tile_pool(name="psAv", bufs=1, space="PSUM") as psAv,
            tc.tile_pool(name="psSa", bufs=2, space="PSUM") as psSa,
        ):
            psS_pool_ref[0] = psSa
            identR = scopeA.tile([P, P], bf16)
            nc.sync.dma_start(identR, id_d.ap())
            wq_sb = scopeA.tile([P, 8, 6 * P], bf16)
            wq_r = wq_d.ap().rearrange("(co p) d -> p co d", p=P)
            xT_tiles = {}

            ld_state = [0]

            def load_chunk(nch, eng):
                ldq = [nc.sync, nc.scalar]
                for co in range(8):
                    xt_c = xtc.tile(
                        [P, QCH], bf16, tag=f"xtc{co}", name=f"xT_{nch}_{co}"
                    )
                    xT_tiles[(nch, co)] = xt_c
                    if eng is None:
                        e = ldq[ld_state[0] % 2]
                        ld_state[0] += 1
                    else:
                        e = eng
                    e.dma_start(
                        xt_c,
                        xT_d[co * P : (co + 1) * P, nch * QCH : (nch + 1) * QCH],
                    )

            # startup: wq + chunk0 on the two hwdge queues (gpsimd DMA
            # enqueue is ~770ns each and would serialize the start)
            ldq = [nc.sync, nc.scalar]
            for co in range(8):
                ldq[co % 2].dma_start(wq_sb[:, co, :], wq_r[:, co, :])
            load_chunk(0, None)
            wp_sb = persist.tile([P, 2, C], bf16)

            done_chunks = [0]

            def avail_early(p, h, k):
                iq, jt = 2 * h + k // NJT, k % NJT
                return jt < 4 * done_chunks[0] and iq < done_chunks[0]

            qk_done = done_chunks

            vT_tiles = {}

            def v_group(nch, u):
                vT = vT_tiles[nch]
                pb = 64 * (u % 2)
                vT_u = vT[pb : pb + 64, u // 2, :]
                ps = psAv.tile([P, 4, HD], bf16, tag="psAv")
                for k in range(4):
                    nc.tensor.transpose(
                        ps[:, k, :],
                        vT_u[:, k * P : (k + 1) * P],
                        identR[pb : pb + 64, pb : pb + 64],
                    )
                nc.vector.tensor_copy(v1[:, u, nch * 4 : (nch + 1) * 4, 0:HD], ps)
                nc.vector.tensor_copy(v1f[:, u, nch * 4 : (nch + 1) * 4, 0:HD], ps)

            for nch in range(NCH):
                if nch + 1 < NCH:
                    load_chunk(nch + 1, None)
                if nch == 1:
                    nc.gpsimd.dma_start(wp_sb, wp_d.ap())
                vT_tiles[nch] = vtc.tile([P, 2, QCH], bf16, tag="vtc", name=f"vT_{nch}")
                vT = vT_tiles[nch]
                sl = slice(nch * QCH, (nch + 1) * QCH)
                for mt in range(6):
                    # two INDEPENDENT half-contraction accumulators in
                    # different PSUM banks; K=64 MMs alternate row-halves so
                    # consecutive MMs hit disjoint row-groups and different
                    # banks (LDW pull-ahead + dual-stream, the S-pair
                    # pattern). Halves are summed by the drain add.
                    psd = [
                        psQ.tile([P, QCH], f32, tag="psQ", name=f"psQ_{nch}_{mt}_{kh}")
                        for kh in range(2)
                    ]
                    for co in range(8):
                        for kh in range(2):
                            pb = 64 * kh
                            nc.tensor.matmul(
                                psd[kh],
                                lhsT=wq_sb[pb : pb + 64, co, mt * P : (mt + 1) * P],
                                rhs=xT_tiles[(nch, co)][pb : pb + 64, :],
                                start=(co == 0),
                                stop=(co == 7),
                            )
                    tmp = xtc.tile([P, QCH], f32, tag="qtmp", name=f"qtmp_{nch}_{mt}")
                    nc.vector.tensor_copy(tmp, psd[1])
                    if mt < 4:
                        # mt 0,1 -> q pair 0,1 ; mt 2,3 -> k pair 0,1
                        nc.vector.tensor_add(qk_sb[:, mt, sl], psd[0], tmp)
                    else:
                        nc.vector.tensor_add(vT[:, mt - 4, :], psd[0], tmp)
                    if nch >= 1 and mt < 4:
                        v_group(nch - 1, mt)
                    if mt == 3:
                        done_chunks[0] = nch + 1
                    try_weave(2, avail_early)
                for co in range(8):
                    del xT_tiles[(nch, co)]
            for u in range(4):
                v_group(NCH - 1, u)
                try_weave(1, avail_early)

        # ======== attention ========
        avail_all = lambda p, h, k: True
        with tc.tile_pool(name="otpool", bufs=1) as otpool:
            OT = otpool.tile([P, 2, nseq], bf16)

            with (
                tc.tile_pool(name="small", bufs=2) as small,
                tc.tile_pool(name="etf00", bufs=2) as etf00,
                tc.tile_pool(name="etf01", bufs=2) as etf01,
                tc.tile_pool(name="etf10", bufs=2) as etf10,
                tc.tile_pool(name="etf11", bufs=2) as etf11,
                tc.tile_pool(name="psS", bufs=2, space="PSUM") as psS,
                tc.tile_pool(name="psO", bufs=4, space="PSUM") as psO,
            ):
                psS_pool_ref[0] = psS
                etf_pools_ref[0] = {
                    (0, 0): etf00, (0, 1): etf01, (1, 0): etf10, (1, 1): etf11
                }

                for p in range(2):
                    for h in range(2):
                        ph = (p, h)
                        for q2 in range(2):
                            iq = 2 * h + q2
                            psO_u = [
                                psO.tile(
                                    [P, 512], f32, tag="psO", name=f"psO_{p}{h}{q2}{u}"
                                )
                                for u in range(2)
                            ]
                            for jt in range(NJT):
                                while cursor[ph] < q2 * NJT + jt + 1:
                                    emit_stile(p, h, cursor[ph], jit=True)
                                try_weave(2, avail_all)
                                for half in range(2):
                                    u = 2 * p + half
                                    et, isjit = et_tiles[(p, jt, iq)]
                                    vv = v1f if isjit else v1
                                    nc.tensor.matmul(
                                        psO_u[half][0:65, :],
                                        lhsT=vv[:, u, jt, 0:65],
                                        rhs=et[:, half * 512 : half * 512 + 512],
                                        start=(jt == 0),
                                        stop=(jt == NJT - 1),
                                    )
                                consumed[ph] = q2 * NJT + jt + 1
                            for jt in range(NJT):
                                et_tiles.pop((p, jt, iq), None)
                            # normalize this quarter; psO rows 0:64 = O'^T,
                            # row 64 = rowsum
                            icols = slice(iq * 512, (iq + 1) * 512)
                            last_q = p == 1 and h == 1 and q2 == 1
                            for half in range(2):
                                pb = 64 * half
                                rs = small.tile([1, 512], f32, tag="rs")
                                nc.vector.tensor_copy(
                                    rs, psO_u[half][HD : HD + 1, :]
                                )
                                recip = small.tile([1, 512], f32, tag="recip")
                                nc.vector.reciprocal_approx_fast(recip, rs)
                                bcast = small.tile([64, 512], f32, tag="bcast")
                                nc.gpsimd.partition_broadcast(bcast, recip)
                                if last_q:
                                    # keep the PE HAM-warm through the final
                                    # drain so proj doesn't start throttled
                                    dmy = psO.tile(
                                        [P, 512], f32, tag="psO",
                                        name=f"dmy_{half}",
                                    )
                                    nc.tensor.matmul(
                                        dmy,
                                        lhsT=bcast[0:64, 0:P],
                                        rhs=bcast[0:64, :],
                                        start=True,
                                        stop=True,
                                    )
                                nc.vector.tensor_mul(
                                    OT[pb : pb + 64, p, icols],
                                    psO_u[half][0:64, :],
                                    bcast,
                                )

            # ======== proj ========
            with (
                tc.tile_pool(name="opool", bufs=4) as opool,
                tc.tile_pool(name="psP", bufs=4, space="PSUM") as psP,
            ):
                pdma = [nc.sync, nc.scalar, nc.gpsimd]
                for it in range(NIT):
                    for ech in range(2):
                        ps = psP.tile([P, ECH], f32, tag="psP", name=f"psP_{it}_{ech}")
                        for co in range(2):
                            nc.tensor.matmul(
                                ps,
                                lhsT=OT[:, co, it * P : (it + 1) * P],
                                rhs=wp_sb[:, co, ech * ECH : (ech + 1) * ECH],
                                start=(co == 0),
                                stop=(co == 1),
                            )
                        ot = opool.tile([P, ECH], bf16, tag="opool", name=f"ot_{it}_{ech}")
                        cp(ot, ps)
                        pdma[(2 * it + ech) % 3].dma_start(
                            out_d[it * P : (it + 1) * P, ech * ECH : (ech + 1) * ECH],
                            ot,
                        )

    nc.compile()
    return nc


def get_nc(nseq=NSEQ):
    if nseq not in _cache:
        _cache[nseq] = _build(nseq)
    return _cache[nseq]


def make_in_maps(x, w_qkv, w_proj, nseq=NSEQ):
    import ml_dtypes

    bf = ml_dtypes.bfloat16
    x = np.ascontiguousarray(x, dtype=np.float32)
    w_qkv = np.ascontiguousarray(w_qkv, dtype=np.float32)
    w_proj = np.ascontiguousarray(w_proj, dtype=np.float32)
    in_maps = []
    xT = [np.ascontiguousarray(x[b, :nseq].T.astype(bf)) for b in range(B)]
    ident = np.eye(P, dtype=np.float32).astype(bf)
    for core in range(8):
        b, hg = core // 4, core % 4
        hs = 4 * hg
        wsel = np.empty((6, P, C), np.float32)
        for mt in range(6):
            t, half = mt // 2, mt % 2
            r0 = t * C + (hs + 2 * half) * HD
            wsel[mt] = w_qkv[r0 : r0 + P, :]
        wqkvT = np.ascontiguousarray(
            wsel.transpose(2, 0, 1).reshape(C, 6 * P).astype(bf)
        )
        wp = np.empty((P, 2, C), np.float32)
        for co in range(2):
            c0 = (hs + 2 * co) * HD
            wp[:, co, :] = w_proj[:, c0 : c0 + P].T
        wp = wp.astype(bf)
        in_maps.append(
            {
                "xT": xT[b],
                "wqkvT": wqkvT,
                "wprojT": wp,
                "ident": ident,
            }
        )
    return in_maps


def kernel(x, w_qkv, w_proj, b_proj):
    from concourse.bass_utils import run_bass_kernel_spmd

    nc = get_nc()
    in_maps = make_in_maps(x, w_qkv, w_proj)
    res = run_bass_kernel_spmd(nc, in_maps, core_ids=list(range(8)))
    parts = [np.asarray(r["out"]).astype(np.float32) for r in res.results]
    out = np.stack(
        [
            parts[0] + parts[1] + parts[2] + parts[3],
            parts[4] + parts[5] + parts[6] + parts[7],
        ],
        axis=0,
    )
    return (out + np.asarray(b_proj, np.float32)).astype(np.float32)
